# revision 20
# baseline (speedup 1.0000x reference)
"""MoE layer (top-2 of 8 experts, D=1024, F=4096) on 8 TRN2 NeuronCores.

Strategy: expert parallel. The gate (scores -> top-2 -> softmax) runs on the
host as part of the sharding step; each core holds one expert's W1/b1/W2/b2
and processes the tokens routed to that expert (gathered + padded to a fixed
capacity on the host). The device runs the FFN as two big matmuls in
float32r (full PE-rate fp32 mode on TRN2):

    hT = relu(W1.T @ xT + b1)     [4096, NT]   (lhsT = W1 [1024, 4096])
    yT = W2.T @ hT + b2           [1024, NT]   (lhsT = W2 [4096, 1024])

Weights are supplied in a host-pre-tiled layout so each SBUF weight slab
loads with a single large DMA (per-DMA overhead on the DGE is ~1.2us, so
few/large transfers matter more than anything else). The host then
scatter-adds prob-weighted per-expert outputs into the full [S, B, D] result.
"""

import numpy as np

D_MODEL = 1024
D_FF = 4096
N_EXPERTS = 8
TOP_K = 2
P = 128
KD = D_MODEL // P    # 8   k-tiles of mm1 (contraction over D)
MF = D_FF // P       # 32  f-tiles (partition tiles of hT; contraction of mm2)
MD = D_MODEL // P    # 8   d-tiles of yT
W1G = 512            # W1 column-group width per SBUF slab
NG1 = D_FF // W1G    # 8   W1 column groups

_CACHE: dict = {}


# ---------------------------------------------------------------- device ----


def _chunk_plan(length):
    """Split `length` into matmul free-dim chunks of <=512, each >=256 (so
    fp32r matmuls run at full PE rate)."""
    chunks = []
    off = 0
    rem = length
    while rem > 0:
        if 512 < rem < 768:
            take = rem - 256
        else:
            take = min(512, rem)
        assert take >= 256, (length, chunks)
        chunks.append((off, take))
        off += take
        rem -= take
    return chunks


def _pass_plan(cap, n_passes):
    """Pass lengths summing to cap.  Make all but the first pass exactly 512
    (a single full-width chunk) and give the remainder to the first pass —
    this minimizes the number of chunk instances (512 matmuls each) while
    keeping the per-pass hT working set bounded."""
    first = cap - 512 * (n_passes - 1)
    if first <= 704:
        lens = [first] + [512] * (n_passes - 1)
    else:
        # bound the per-pass hT working set (SBUF): balance the passes
        assert n_passes == 2
        a = -(-cap // 32) * 16
        lens = [a, cap - a]
    assert all(256 <= l <= 704 for l in lens), lens
    return lens


def _build(cap, n_passes, h_bf16):
    """Build the SPMD single-core program: one expert FFN over `cap` tokens."""
    import concourse.mybir as mybir
    import concourse.tile as tile
    from concourse import bacc

    f32 = mybir.dt.float32
    f32r = mybir.dt.float32r
    h_dt = mybir.dt.bfloat16 if h_bf16 else f32r

    nc = bacc.Bacc("TRN2", target_bir_lowering=False, debug=False)

    xT = nc.dram_tensor("xT", [D_MODEL, cap], f32r, kind="ExternalInput").ap()
    # host-tiled weights, stored exactly in SBUF slab order (see kernel()):
    # w1t[g, p, kd*W1G + w] = W1[kd*128 + p, g*W1G + w]
    # w2t[md, p, mf*128 + c] = W2[mf*128 + p, md*128 + c]
    w1t = nc.dram_tensor("w1t", [NG1, P, KD * W1G], f32r,
                         kind="ExternalInput").ap()
    w2t = nc.dram_tensor("w2t", [MD, P, MF * P], h_dt,
                         kind="ExternalInput").ap()
    b1s = nc.dram_tensor("b1s", [P, MF], f32, kind="ExternalInput").ap()
    b2s = nc.dram_tensor("b2s", [P, MD], f32, kind="ExternalInput").ap()
    yT = nc.dram_tensor("yT", [D_MODEL, cap], f32, kind="ExternalOutput").ap()

    pass_lens = _pass_plan(cap, n_passes)

    with tile.TileContext(nc) as tc:
        with (
            tc.tile_pool(name="const", bufs=1) as const,
            tc.tile_pool(name="xp", bufs=1) as xp,
            tc.tile_pool(name="w1p", bufs=2) as w1p,
            tc.tile_pool(name="w2p", bufs=2) as w2p,
            tc.tile_pool(name="hp", bufs=1) as hp,
            tc.tile_pool(name="yp", bufs=2) as yp,
            tc.tile_pool(name="ps1", bufs=4, space="PSUM") as ps1p,
            tc.tile_pool(name="ps2", bufs=4, space="PSUM") as ps2p,
        ):
            b1_sb = const.tile([P, MF], f32, tag="b1")
            nc.sync.dma_start(b1_sb[:], b1s[:, :])
            b2_sb = const.tile([P, MD], f32, tag="b2")
            nc.sync.dma_start(b2_sb[:], b2s[:, :])

            # load the first chunk's x columns + the first W1 slab before the
            # bulk of x, so mm1 starts as early as possible
            first_clen = _chunk_plan(pass_lens[0])[0][1]
            x_sb = []
            for kd in range(KD):
                t = xp.tile([P, cap], f32r, tag=f"x{kd}")
                nc.sync.dma_start(t[:, :first_clen],
                                  xT[kd * P:(kd + 1) * P, :first_clen])
                x_sb.append(t)
            w1_sb0 = w1p.tile([P, KD * W1G], f32r, tag="w1")
            nc.sync.dma_start(w1_sb0[:], w1t[0])
            for kd in range(KD):
                nc.sync.dma_start(x_sb[kd][:, first_clen:],
                                  xT[kd * P:(kd + 1) * P, first_clen:])

            poff = 0
            for pass_len in pass_lens:
                chunks = _chunk_plan(pass_len)

                # ---- mm1: hT[4096, pass_len] = relu(W1.T @ xT + b1) ----
                # W1 slab g: [128, KD * W1G], slab[:, kd*W1G + w] =
                # W1[kd*128 + p, g*W1G + w] -> one contiguous-src DMA.
                h_tiles = [[None] * MF for _ in chunks]
                for g in range(NG1):
                    if w1_sb0 is not None and g == 0:
                        w1_sb, w1_sb0 = w1_sb0, None
                    else:
                        w1_sb = w1p.tile([P, KD * W1G], f32r, tag="w1")
                        nc.sync.dma_start(w1_sb[:], w1t[g])
                    for ms in range(W1G // P):
                        mf = g * (W1G // P) + ms
                        for ci, (coff, clen) in enumerate(chunks):
                            ps = ps1p.tile([P, clen], f32, tag="ps1")
                            for kd in range(KD):
                                nc.tensor.matmul(
                                    ps[:],
                                    w1_sb[:, kd * W1G + ms * P:
                                          kd * W1G + (ms + 1) * P],
                                    x_sb[kd][:, poff + coff:poff + coff + clen],
                                    start=(kd == 0), stop=(kd == KD - 1))
                            h = hp.tile([P, clen], h_dt, tag=f"h{mf}_{ci}")
                            nc.scalar.activation(
                                h[:], ps[:],
                                mybir.ActivationFunctionType.Relu,
                                bias=b1_sb[:, mf:mf + 1])
                            h_tiles[ci][mf] = h

                # ---- mm2: yT[1024, pass_len] = W2.T @ hT + b2 ----
                # W2 slab md: [128, MF * P], slab[:, mf*P + c] =
                # W2[mf*128 + p, md*128 + c] -> one contiguous-src DMA.
                for md in range(MD):
                    w2_sb = w2p.tile([P, MF * P], h_dt, tag="w2")
                    nc.sync.dma_start(w2_sb[:], w2t[md])
                    for ci, (coff, clen) in enumerate(chunks):
                        ps = ps2p.tile([P, clen], f32, tag="ps2")
                        for mf in range(MF):
                            nc.tensor.matmul(
                                ps[:],
                                w2_sb[:, mf * P:(mf + 1) * P],
                                h_tiles[ci][mf][:],
                                start=(mf == 0), stop=(mf == MF - 1))
                        y = yp.tile([P, clen], f32, tag=f"y{md % 2}")
                        nc.scalar.activation(
                            y[:], ps[:],
                            mybir.ActivationFunctionType.Identity,
                            bias=b2_sb[:, md:md + 1])
                        nc.sync.dma_start(
                            yT[md * P:(md + 1) * P,
                               poff + coff:poff + coff + clen], y[:])
                poff += pass_len

    nc.compile()
    return nc


def _get_program(cap, n_passes, h_bf16):
    key = (cap, n_passes, h_bf16)
    if key not in _CACHE:
        _CACHE[key] = _build(cap, n_passes, h_bf16)
    return _CACHE[key]


# ------------------------------------------------------------------ host ----


VARIANT_H_BF16 = False   # False: all-float32r (2 passes); True: bf16 h/W2
N_PASSES = 2


def kernel(x, gate_w, gate_b, w1, b1, w2, b2):
    from concourse import bass_utils

    S, B, D = x.shape
    N = S * B
    x = np.ascontiguousarray(np.asarray(x, dtype=np.float32))
    x_flat = x.reshape(N, D)

    # --- gate (host, fp64 for a faithful top-k) ---
    scores = x_flat.astype(np.float64) @ np.asarray(gate_w, np.float64)
    scores += np.asarray(gate_b, np.float64)
    order = np.argsort(-scores, axis=1, kind="stable")
    top_idx = order[:, :TOP_K]                       # [N, K]
    top_val = np.take_along_axis(scores, top_idx, axis=1)
    top_val -= top_val.max(axis=1, keepdims=True)
    e_val = np.exp(top_val)
    probs = (e_val / e_val.sum(axis=1, keepdims=True)).astype(np.float32)

    # --- gather per expert ---
    idx_e = [np.where((top_idx == e).any(axis=1))[0] for e in range(N_EXPERTS)]
    p_e = []
    for e in range(N_EXPERTS):
        sel = (top_idx[idx_e[e]] == e)
        p_e.append((probs[idx_e[e]] * sel).sum(axis=1))
    max_count = max(len(i) for i in idx_e)

    # One device call handles up to 1280 tokens per expert (2 passes of
    # <=768/512).  Heavier routing imbalance (never seen with the spec's
    # input distribution) falls back to multiple device calls.
    n_passes = N_PASSES
    batch_cap = 1280
    if max_count <= batch_cap:
        n_batches = 1
        cap = max(768, -(-max_count // 16) * 16)
    else:
        n_batches = -(-max_count // batch_cap)
        cap = batch_cap

    nc = _get_program(cap, n_passes, VARIANT_H_BF16)

    w1 = np.asarray(w1, np.float32)
    b1 = np.asarray(b1, np.float32)
    w2 = np.asarray(w2, np.float32)
    b2 = np.asarray(b2, np.float32)
    if VARIANT_H_BF16:
        import ml_dtypes
        w2 = w2.astype(ml_dtypes.bfloat16)

    base_maps = []
    for e in range(N_EXPERTS):
        # w1t[g, p, kd*W1G + w] = W1[kd*128 + p, g*W1G + w]
        w1t = np.ascontiguousarray(
            w1[e].reshape(KD, P, NG1, W1G).transpose(2, 1, 0, 3)
        ).reshape(NG1, P, KD * W1G)
        # w2t[md, p, mf*128 + c] = W2[mf*128 + p, md*128 + c]
        w2t = np.ascontiguousarray(
            w2[e].reshape(MF, P, MD, P).transpose(2, 1, 0, 3)
        ).reshape(MD, P, MF * P)
        base_maps.append({
            "w1t": w1t,
            "w2t": w2t,
            "b1s": np.ascontiguousarray(b1[e].reshape(MF, P).T),
            "b2s": np.ascontiguousarray(b2[e].reshape(MD, P).T),
        })

    out = np.zeros((N, D), np.float32)
    for b in range(n_batches):
        in_maps = []
        for e in range(N_EXPERTS):
            idx_b = idx_e[e][b * cap:(b + 1) * cap]
            xT_e = np.zeros((D, cap), np.float32)
            xT_e[:, :len(idx_b)] = x_flat[idx_b].T
            in_maps.append({"xT": xT_e, **base_maps[e]})
        res = bass_utils.run_bass_kernel_spmd(
            nc, in_maps, core_ids=list(range(N_EXPERTS)))
        for e in range(N_EXPERTS):
            idx_b = idx_e[e][b * cap:(b + 1) * cap]
            p_b = p_e[e][b * cap:(b + 1) * cap]
            y_e = res.results[e]["yT"][:, :len(idx_b)].T   # [cnt, D]
            out[idx_b] += p_b[:, None] * y_e               # idx_b is unique

    return out.reshape(S, B, D)
